# revision 4
# baseline (speedup 1.0000x reference)
"""Trainium2 Bass kernel for nn_NoiseProjector — symmetric-fold fc version.

Strategy (8 NeuronCores):
- Data-parallel conv trunk (unchanged from baseline): each core runs
  conv1+conv2+GAP on 8 of 64 images; AllGather of pooled features.
- fc stage: exploits the full S3 symmetry of fc⊗fc⊗fc.  The third-order
  weights are folded host-side onto unique multisets {i<=j<=k}:
    third[b,o] = sum_i fc_i * sum_{(j<=k), j>=i} S[(j,k),(o,i)] * fc_j*fc_k
  with S = symmetrized w3 (each multiset counted once).  Pairs are ordered
  by j DESCENDING so for every i-block the needed pairs are a prefix; the
  w3s stream is blocked over 16 i-groups of 4, upper-triangular, cutting
  HBM traffic from 33.5 MB (fp8 baseline) to ~7 MB per core.
- covuT[(j,k), b] = (4fc_j)(4fc_k) built on-device via one-hot gather
  matmuls + DVE multiply, stored fp8; the w3s stream is consumed with
  fp8 DoubleRow matmuls (2 k-tiles per instruction, 0.5 cyc/row).
- cov (wc) and mean (wm) terms reuse covuT / a feat transpose; all biases
  are folded into an extra ones-row of the wm matmul (host-side).
"""

import sys

sys.path.insert(0, "/opt/trn_rl_repo")

import numpy as np
import ml_dtypes

B = 64          # global batch
BL = 8          # images per core
NCORES = 8
OPC = 128       # outputs per core
FEAT = 64
H, W = 224, 224
H1, W1 = 112, 112   # conv1 out
H2, W2 = 56, 56     # conv2 out
YC_HOST = 8         # conv1 y-rows per chunk (host pre-tiled layout)
GAP = 1.0 / (H2 * W2)

# ---- symmetric pair enumeration: j descending, k ascending ----
_JJ = np.array([j for j in range(63, -1, -1) for k in range(j, 64)])
_KK = np.array([k for j in range(63, -1, -1) for k in range(j, 64)])
NPAIR = 2080
NPT = 17                     # pair tiles of 128 (padded)
GSZ = 4                      # i-group size
NG = 16                      # number of i-groups
KG = [(64 - GSZ * g) * (65 - GSZ * g) // 2 for g in range(NG)]
NTILE = [-(-k // 128) for k in KG]        # [17,15,13,11,10,8,7,6,5,4,3,2,2,1,1,1]
NDR = [n // 2 for n in NTILE]             # DoubleRow blocks per group
TAIL = [n % 2 for n in NTILE]             # regular tail tile per group
NDRT = sum(NDR)                           # 48
NTAILT = sum(TAIL)                        # 10
FC_SCALE = 4.0                            # fc scaled by 4 -> products x16
PWC_SCALE = 16.0                          # wm/bias host-side scale


def _split_multiwait_json(raw):
    """This walrus build accepts only ONE sync wait per instruction.  Split any
    multi-wait instruction into single-wait EventSemaphore ops ahead of it (the
    engine is in-order, so chained waits are equivalent)."""
    import json

    j = json.loads(raw)
    n_split = 0
    for f in j["functions"]:
        for bb in f["blocks"]:
            insts = bb.get("instructions")
            if not insts:
                continue
            out = []
            changed = False
            for ins in insts:
                si = ins.get("sync_info")
                waits = si.get("on_wait") if si else None
                if waits and len(waits) > 1:
                    changed = True
                    keep = None
                    for w in waits:
                        if w.get("wait_reg") is not None:
                            keep = w
                    if keep is None:
                        keep = waits[-1]
                    rest = [w for w in waits if w is not keep]
                    for k, w in enumerate(rest):
                        n_split += 1
                        out.append({
                            "engine": ins["engine"], "ins": [], "outs": [],
                            "name": f"{ins['name']}-sw{k}",
                            "opcode": "EventSemaphore",
                            "sync_info": {"on_update": [], "on_wait": [w]},
                        })
                    si["on_wait"] = [keep]
                out.append(ins)
            if changed:
                bb["instructions"] = out
    return json.dumps(j).encode(), n_split


def _build(reps=1, trivial=False, conv_reps=1, fc_reps_extra=0, w3bufs=58):
    YC = YC_HOST
    NQ = H1 // YC
    import concourse.bass as bass
    import concourse.mybir as mybir
    import concourse.tile as tile
    from concourse.masks import make_identity

    F32, F16, BF16 = mybir.dt.float32, mybir.dt.float16, mybir.dt.bfloat16
    F8 = mybir.dt.float8e4
    DRMODE = mybir.MatmulPerfMode.DoubleRow
    AF = mybir.ActivationFunctionType
    ALU = mybir.AluOpType
    AX = mybir.AxisListType

    nc = bass.Bass("TRN2", target_bir_lowering=False, num_devices=NCORES)

    xb3 = nc.dram_tensor(
        "xb3", (H1 // YC_HOST, 2, 27, 4, YC_HOST, W + 2), BF16,
        kind="ExternalInput").ap()
    w1t = nc.dram_tensor("w1t", (27, 32), BF16, kind="ExternalInput").ap()
    b1 = nc.dram_tensor("b1", (32, 1), F32, kind="ExternalInput").ap()
    w2t = nc.dram_tensor("w2t", (32, 3, 3, 64), BF16, kind="ExternalInput").ap()
    b2 = nc.dram_tensor("b2", (64, 1), F32, kind="ExternalInput").ap()
    wmx = nc.dram_tensor("wmx", (65, OPC), F32, kind="ExternalInput").ap()
    wcs = nc.dram_tensor("wcs", (128, NPT, OPC), F8, kind="ExternalInput").ap()
    sel = nc.dram_tensor("sel", (64, 2, NPT, 128), F8, kind="ExternalInput").ap()
    w3d = nc.dram_tensor("w3d", (NDRT, 128, 2, 512), F8, kind="ExternalInput").ap()
    w3x = nc.dram_tensor("w3x", (NTAILT, 128, 512), F8, kind="ExternalInput").ap()
    out = nc.dram_tensor("out", (B, OPC), F32, kind="ExternalOutput").ap()
    feat_loc = nc.dram_tensor("feat_loc", (BL, FEAT), F32).ap()
    feat_all = nc.dram_tensor("feat_all", (B, FEAT), F32, addr_space="Shared").ap()

    if trivial:
        with tile.TileContext(nc) as tc:
            with tc.tile_pool(name="tp", bufs=1) as tp:
                z = tp.tile([B, OPC], F32)
                nc.vector.memset(z[:], 0.0)
                nc.sync.dma_start(out[:], z[:])
        nc.finalize()
        fixed, _ = _split_multiwait_json(nc.to_json_bytes())
        nc.to_json_bytes = lambda: fixed
        return nc

    with tile.TileContext(nc) as tc:
        with (
            tc.tile_pool(name="w3pool", bufs=w3bufs) as w3pool,
            tc.tile_pool(name="w3tpool", bufs=10) as w3tpool,
            tc.tile_pool(name="consts", bufs=1) as consts,
            tc.tile_pool(name="fcsingle", bufs=1) as fcsingle,
            tc.tile_pool(name="fcwork", bufs=2) as fcwork,
        ):
            # ---- constants ----
            w1sb = consts.tile([64, 32], BF16)          # [(32r)+(dx,dy,ci), o]
            for r in range(2):
                nc.sync.dma_start(w1sb[32 * r:32 * r + 27, :], w1t[:])
            w2sb = consts.tile([128, 3, 3, 64], BF16)   # [(32c)+ci, dy, dx, o]
            for c in range(4):
                nc.sync.dma_start(w2sb[32 * c:32 * c + 32], w2t[:])
            bias1 = consts.tile([128, 1], F32)          # b1[cout] at 32c+cout
            nc.sync.dma_start(
                bias1[:],
                bass.AP(tensor=b1.tensor, offset=0, ap=[[0, 4], [1, 32], [1, 1]]),
            )
            bias2 = consts.tile([128, 1], F32)       # b2[co] at (r, co)
            nc.sync.dma_start(
                bias2[:],
                bass.AP(tensor=b2.tensor, offset=0, ap=[[0, 2], [1, 64], [1, 1]]),
            )
            wmxsb = consts.tile([65, OPC], F32)
            nc.sync.dma_start(wmxsb[:], wmx[:])
            wcssb = consts.tile([128, NPT, OPC], F8)
            nc.sync.dma_start(wcssb[:], wcs[:])
            selsb = consts.tile([64, 2, NPT, 128], F8)
            nc.sync.dma_start(selsb[:], sel[:])
            ident = consts.tile([64, 64], F32)
            make_identity(nc, ident[:])
            zeros = consts.tile([128, 448], BF16)
            nc.vector.memset(zeros[:], 0.0)
            featparts = consts.tile([128, 4, 7], F32)   # (r, co) x (c, sc)

            # =============== conv trunk (unchanged from baseline) ===========
            with (
                tc.tile_pool(name="conv", bufs=2) as conv,
                tc.tile_pool(name="h1p", bufs=1) as h1p,
            ):
                h1 = h1p.tile([128, 2, H1 + 2, W1 + 2], BF16)
                nc.vector.memset(h1[:, :, 0:1, :], 0.0)
                nc.vector.memset(h1[:, :, :, 0:1], 0.0)

                for _crep in range(conv_reps):
                    cpsum_cm = tc.tile_pool(name="cpsum", bufs=3, space="PSUM")
                    cpsum = cpsum_cm.__enter__()
                    for q in range(NQ):
                        a1 = conv.tile([64, 4, YC, W + 2], BF16, tag="a1")
                        for r in range(2):
                            nc.sync.dma_start(
                                a1[32 * r:32 * r + 27, :, :, :], xb3[q, r])
                        for s in range(YC // 4):
                            ps1 = cpsum.tile([128, 2, 512], F32, tag="cpsum")
                            for r in range(2):
                                for c in range(4):
                                    rhs = a1[32 * r:32 * r + 27, c,
                                             4 * s:4 * s + 4, 0:2 * W1:2]
                                    nc.tensor.matmul(
                                        ps1[32 * c:32 * c + 32, r, 0:448],
                                        w1sb[32 * r:32 * r + 27, :],
                                        rhs,
                                        start=True, stop=True,
                                        tile_position=(32 * r, 32 * c),
                                        skip_group_check=True,
                                    )
                            ybase = 1 + q * YC + 4 * s
                            for r in range(2):
                                src = ps1[:, r, 0:448].rearrange("p (y x) -> p y x", y=4)
                                dst = h1[:, r, ybase:ybase + 4, 1:113]
                                if r == 0:
                                    nc.scalar.activation(dst, src, AF.Relu,
                                                         bias=bias1[:], scale=1.0)
                                else:
                                    nc.vector.scalar_tensor_tensor(
                                        dst, src, bias1[:],
                                        zeros[:].rearrange("p (y x) -> p y x", y=4),
                                        op0=ALU.add, op1=ALU.max,
                                    )

                    cpsum_cm.__exit__(None, None, None)

                    with tc.tile_pool(name="c2psum", bufs=2, space="PSUM") as c2psum:
                        trash = consts.tile([128, 448], BF16)
                        trash2 = consts.tile([128, 448], BF16)
                        for sc in range(7):
                            ps2 = c2psum.tile([128, 4, 512], F32, tag="c2psum")
                            for dy in range(3):
                                for dx in range(3):
                                    for c in range(4):
                                        for r in range(2):
                                            rhs = h1[32 * c:32 * c + 32, r,
                                                     2 * (8 * sc) + dy:2 * (8 * sc) + dy + 16:2,
                                                     dx:dx + 2 * W2:2]
                                            nc.tensor.matmul(
                                                ps2[64 * r:64 * r + 64, c, 0:448],
                                                w2sb[32 * c:32 * c + 32, dy, dx, :],
                                                rhs,
                                                start=(dy == 0 and dx == 0),
                                                stop=(dy == 2 and dx == 2),
                                                tile_position=(32 * c, 64 * r),
                                                skip_group_check=True,
                                            )
                            for c in range(4):
                                src = ps2[:, c, 0:448]
                                acc_o = featparts[:, c, sc:sc + 1]
                                if c % 2 == 0:
                                    nc.scalar.activation(
                                        trash[:], src, AF.Relu,
                                        bias=bias2[:],
                                        scale=1.0, accum_out=acc_o,
                                    )
                                else:
                                    nc.vector.scalar_tensor_tensor(
                                        trash2[:], src, bias2[:],
                                        zeros[:],
                                        op0=ALU.add, op1=ALU.max,
                                        accum_out=acc_o,
                                    )

            # featTl2[(r, f), c] -> feat_loc[2c+r, f]
            featTl2 = fcsingle.tile([128, 4], F32, tag="featTl2")
            nc.vector.tensor_reduce(featTl2[:], featparts[:], AX.X, op=ALU.add)
            nc.vector.tensor_scalar_mul(featTl2[:], featTl2[:], GAP)
            nc.sync.dma_start(
                bass.AP(tensor=feat_loc.tensor, offset=0,
                        ap=[[64, 2], [1, 64], [128, 4]]),
                featTl2[:])

            nc.gpsimd.collective_compute(
                "AllGather", ALU.bypass,
                replica_groups=[list(range(NCORES))],
                ins=[feat_loc[:]], outs=[feat_all[:]],
            )

            for _rep in range(reps):
                # =============== fc prep ===============
                feat = fcsingle.tile([64, 64], F32, tag="feat")
                nc.sync.dma_start(feat[:], feat_all[:])
                mean = fcsingle.tile([64, 1], F32, tag="mean")
                nc.vector.tensor_reduce(mean[:], feat[:], AX.X, op=ALU.add)
                nc.vector.tensor_scalar_mul(mean[:], mean[:], 1.0 / FEAT)
                fc = fcsingle.tile([64, 64], F32, tag="fc")
                nc.vector.tensor_scalar_sub(fc[:], feat[:], mean[:])

                featTx = fcsingle.tile([65, 64], F32, tag="featTx")
                nc.vector.memset(featTx[64:65, :], 1.0)
                fcT4 = fcsingle.tile([64, 64], BF16, tag="fcT4")
                covuT = fcsingle.tile([128, (NPT + 1) // 2, 2, 64], F8, tag="covuT")
                thirdparts = fcsingle.tile([64, OPC, NG], F32, tag="thp")

                with tc.tile_pool(name="wpsum", bufs=1, space="PSUM") as wpsum:
                    pwc = wpsum.tile([64, OPC], F32)
                    with tc.tile_pool(name="tpsum", bufs=2, space="PSUM") as tpsum, \
                         tc.tile_pool(name="gpsum", bufs=3, space="PSUM") as gpsum:
                        # transposes: feat -> featTx[0:64];  fc*4 -> fcT4
                        pT = tpsum.tile([64, 64], F32, tag="pT")
                        nc.tensor.transpose(pT[:], feat[:], ident[:])
                        nc.vector.tensor_copy(featTx[0:64, :], pT[:])
                        pT2 = tpsum.tile([64, 64], F32, tag="pT")
                        nc.tensor.transpose(pT2[:], fc[:], ident[:])
                        nc.vector.tensor_scalar_mul(fcT4[:], pT2[:], FC_SCALE)

                        # covuT build: per pair-tile, gather j/k rows, multiply
                        # (ACT copies gather-A to SBUF: HW allows only one
                        #  PSUM operand per vector instruction)
                        for t in range(NPT):
                            psg = gpsum.tile([128, 2, 64], F32, tag="psg")
                            nc.tensor.matmul(psg[:, 0, :], selsb[:, 0, t, :],
                                             fcT4[:], start=True, stop=True,
                                             skip_group_check=True)
                            nc.tensor.matmul(psg[:, 1, :], selsb[:, 1, t, :],
                                             fcT4[:], start=True, stop=True,
                                             skip_group_check=True)
                            ga = fcwork.tile([128, 64], BF16, tag="ga")
                            nc.scalar.activation(ga[:], psg[:, 0, :], AF.Copy)
                            nc.vector.tensor_mul(covuT[:, t // 2, t % 2, :],
                                                 psg[:, 1, :], ga[:])

                        # pwc = 16*(mean_feat + cov_feat + biases)   [64, 128]
                        nc.tensor.matmul(pwc[:], featTx[:], wmxsb[:],
                                         start=True, stop=False)
                        for t in range(NPT // 2):
                            nc.tensor.matmul(pwc[:], covuT[:, t, :, :],
                                             wcssb[:, 2 * t:2 * t + 2, :],
                                             start=False, stop=False,
                                             perf_mode=DRMODE)
                        nc.tensor.matmul(pwc[:], covuT[:, NPT // 2, 0, :],
                                         wcssb[:, NPT - 1, :],
                                         start=False, stop=True)

                    # ---- big w3s stream: 16 upper-triangular i-groups ----
                    with tc.tile_pool(name="fpsum", bufs=3, space="PSUM") as fpsum:
                        for _ex in range(fc_reps_extra + 1):
                            bd = 0
                            bx = 0
                            for t in range(NG // 2):
                                pg = fpsum.tile([64, 2, 512], F32, tag="pg")
                                for h in range(2):
                                    g = 2 * t + h
                                    for dr in range(NDR[g]):
                                        w3blk = w3pool.tile([128, 2, 512], F8,
                                                            tag="w3d")
                                        nc.sync.dma_start(w3blk[:], w3d[bd])
                                        bd += 1
                                        nc.tensor.matmul(
                                            pg[:, h, :], covuT[:, dr, :, :],
                                            w3blk[:],
                                            start=(dr == 0),
                                            stop=(dr == NDR[g] - 1
                                                  and not TAIL[g]),
                                            perf_mode=DRMODE,
                                            skip_group_check=True)
                                    if TAIL[g]:
                                        w3tl = w3tpool.tile([128, 512], F8,
                                                            tag="w3x")
                                        nc.sync.dma_start(w3tl[:], w3x[bx])
                                        bx += 1
                                        lastt = NTILE[g] - 1
                                        nc.tensor.matmul(
                                            pg[:, h, :],
                                            covuT[:, lastt // 2, lastt % 2, :],
                                            w3tl[:],
                                            start=(NDR[g] == 0), stop=True,
                                            skip_group_check=True)
                                # final contraction over i (4 per group)
                                tmp = fcwork.tile([64, 2, OPC, GSZ], F32,
                                                  tag="tmp")
                                for h in range(2):
                                    g = 2 * t + h
                                    nc.vector.tensor_mul(
                                        tmp[:, h],
                                        pg[:, h, :].rearrange(
                                            "b (o i) -> b o i", o=OPC),
                                        fc[:, GSZ * g:GSZ * (g + 1)]
                                            .unsqueeze(1)
                                            .broadcast_to([64, OPC, GSZ]),
                                    )
                                    nc.vector.tensor_reduce(
                                        thirdparts[:, :, g], tmp[:, h],
                                        AX.X, op=ALU.add)

                        # ---- final assembly ----
                        tr3 = fcsingle.tile([64, OPC], F32, tag="tr3")
                        nc.vector.tensor_reduce(tr3[:], thirdparts[:], AX.X,
                                                op=ALU.add)
                        acc = fcsingle.tile([64, OPC], F32, tag="acc")
                        nc.vector.scalar_tensor_tensor(
                            acc[:], pwc[:], 1.0, tr3[:],
                            op0=ALU.bypass, op1=ALU.add)
                        nc.vector.tensor_scalar_mul(acc[:], acc[:],
                                                    1.0 / PWC_SCALE)
                        nc.sync.dma_start(out[:], acc[:])

    nc.finalize()
    fixed, n_split = _split_multiwait_json(nc.to_json_bytes())
    nc.to_json_bytes = lambda: fixed
    return nc


_NC_CACHE = None


def _get_nc():
    global _NC_CACHE
    if _NC_CACHE is None:
        _NC_CACHE = _build()
    return _NC_CACHE


def _prep_conv(inputs):
    x = np.asarray(inputs["x"])
    w1 = np.asarray(inputs["w1"])
    b1 = np.asarray(inputs["b1"])
    w2 = np.asarray(inputs["w2"])
    b2 = np.asarray(inputs["b2"])
    bf16 = ml_dtypes.bfloat16
    NQH = H1 // YC_HOST
    # 27-tap pre-fold: tap (dx,dy,ci) at padded row 2*y1+dy, col u+dx
    xpad = np.zeros((B, 3, H + 2, W + 4), np.float32)
    xpad[:, :, 1:H + 1, 1:W + 1] = x
    yidx = 2 * np.arange(H1)[:, None] + np.arange(3)[None, :]   # (112, 3)
    g = xpad[:, :, yidx, :]                      # (img, ci, y1, dy, u228)
    arr = np.stack([g[..., d:d + W + 2] for d in range(3)], axis=1)
    # (img, dx, ci, y1, dy, u) -> (img, dx, dy, ci, y1, u)
    arr = arr.transpose(0, 1, 4, 2, 3, 5).reshape(B, 27, H1, W + 2)
    # xb3[n, q, r, t, c, y, u] = arr[8n + 2c + r, t, 8q+y, u]
    a6 = arr.reshape(NCORES, 4, 2, 27, NQH, YC_HOST, W + 2)
    xb3 = np.ascontiguousarray(
        a6.transpose(0, 4, 2, 3, 1, 5, 6)).astype(bf16)
    w1t = np.ascontiguousarray(
        w1.transpose(3, 2, 1, 0).reshape(27, 32)).astype(bf16)
    w2t = np.ascontiguousarray(w2.transpose(1, 2, 3, 0)).astype(bf16)
    b1r = np.ascontiguousarray(b1.reshape(32, 1)).astype(np.float32)
    b2r = np.ascontiguousarray(b2.reshape(64, 1)).astype(np.float32)
    return xb3, w1t, w2t, b1r, b2r


def _prep_w3s_core(w3c):
    """w3c: (OPC, 64^3) f32 for one core -> (w3d, w3x) fp8 stream blocks."""
    f8 = ml_dtypes.float8_e4m3
    Wt = w3c.reshape(OPC, 64, 64, 64)
    T = (Wt
         + Wt.transpose(0, 1, 3, 2)
         + Wt.transpose(0, 2, 1, 3)
         + Wt.transpose(0, 2, 3, 1)
         + Wt.transpose(0, 3, 1, 2)
         + Wt.transpose(0, 3, 2, 1))
    drs = []
    tails = []
    for g in range(NG):
        iv = GSZ * g + np.arange(GSZ)
        Kg = KG[g]
        nt = NTILE[g]
        jjg = _JJ[:Kg]
        kkg = _KK[:Kg]
        sub = T[:, iv][:, :, jjg, kkg]                   # (o, i, p)
        i_ = iv[None, :, None]
        j_ = jjg[None, None, :]
        k_ = kkg[None, None, :]
        eq_ij = (i_ == j_)
        eq_jk = (j_ == k_)
        fac = np.where(eq_ij & eq_jk, 1.0 / 6.0,
                       np.where(eq_ij | eq_jk, 0.5, 1.0))
        mask = (j_ >= i_)
        vals = sub * (fac * mask)
        block = np.zeros((nt * 128, OPC, GSZ), np.float32)
        block[:Kg] = vals.transpose(2, 0, 1)             # (p, o, i)
        block = block.reshape(nt, 128, OPC * GSZ)
        for t in range(nt // 2):
            a = block[2 * t][:, None, :]
            bb = block[2 * t + 1][:, None, :]
            drs.append(np.concatenate([a, bb], axis=1))  # (128, 2, 512)
        if nt % 2:
            tails.append(block[nt - 1])
    return (np.stack(drs).astype(f8), np.stack(tails).astype(f8))


def _prepare_in_maps(inputs):
    f8 = ml_dtypes.float8_e4m3
    xb3, w1t, w2t, b1r, b2r = _prep_conv(inputs)
    wm = np.asarray(inputs["wm"])
    bm = np.asarray(inputs["bm"])
    wc = np.asarray(inputs["wc"])
    bc = np.asarray(inputs["bc"])
    w3 = np.asarray(inputs["w3"])
    b3 = np.asarray(inputs["b3"])

    # selection matrices (shared across cores)
    selA = np.zeros((64, NPT * 128), np.float32)
    selB = np.zeros((64, NPT * 128), np.float32)
    selA[_JJ, np.arange(NPAIR)] = 1.0
    selB[_KK, np.arange(NPAIR)] = 1.0
    sel = np.stack([selA.reshape(64, NPT, 128),
                    selB.reshape(64, NPT, 128)], axis=1).astype(f8)

    in_maps = []
    for c in range(NCORES):
        osl = slice(OPC * c, OPC * (c + 1))
        # wm + all biases (x16)
        wmx = np.zeros((65, OPC), np.float32)
        wmx[0:64] = wm[osl].T * PWC_SCALE
        wmx[64] = (bm[osl] + bc[osl] + b3[osl]) * PWC_SCALE
        # symmetrized wc in pair order
        wcc = wc[osl].reshape(OPC, 64, 64)
        wcp = wcc[:, _JJ, _KK] + np.where(_JJ != _KK, 1.0, 0.0) * wcc[:, _KK, _JJ]
        wcs = np.zeros((NPT * 128, OPC), np.float32)
        wcs[:NPAIR] = wcp.T
        wcs = np.ascontiguousarray(
            wcs.reshape(NPT, 128, OPC).transpose(1, 0, 2)).astype(f8)
        w3dc, w3xc = _prep_w3s_core(w3[osl])
        in_maps.append({
            "xb3": np.ascontiguousarray(xb3[c]),
            "w1t": w1t,
            "b1": b1r,
            "w2t": w2t,
            "b2": b2r,
            "wmx": wmx,
            "wcs": wcs,
            "sel": sel,
            "w3d": w3dc,
            "w3x": w3xc,
        })

    return in_maps


def kernel(**inputs):
    in_maps = _prepare_in_maps(inputs)
    from concourse.bass_utils import run_bass_kernel_spmd

    res = run_bass_kernel_spmd(_get_nc(), in_maps, core_ids=list(range(NCORES)))
    return np.concatenate([res.results[c]["out"] for c in range(NCORES)], axis=1)


if __name__ == "__main__":
    nc = _build()
    print("built OK; instructions:",
          sum(len(bb.instructions) for f in nc.m.functions for bb in f.blocks))
